# revision 1
# baseline (speedup 1.0000x reference)
"""Trainium2 Bass kernel for nn_MicroBiMambaBackbone.

Strategy: data-parallel over batch (B=8 -> 8 cores, 1 sample/core).
Per core, activations live in (feature-partition, L-free) layout:
  - residual h: (64, L) f32
  - inner activations: (128, L)
The selective-scan recurrence runs as 16 `tensor_tensor_scan` ops
(one per state index n) over the full L free dim.  Bm/Cm coefficient
broadcasts across the 128 d-partitions are produced directly by PE
matmuls with host-built replicated-column weights.  The depthwise
causal conv runs on PE as 4 accumulating diagonal matmuls over a
zero-padded input.  LayerNorm stats use gpsimd partition_all_reduce.
All weights are pre-packed host-side (bf16 where harmless, ln_w/ln_b
and the 1/L mean scale folded in).
"""

import sys

sys.path.insert(0, "/opt/trn_rl_repo")

from contextlib import ExitStack

import ml_dtypes
import numpy as np

import concourse.bacc as bacc
import concourse.bass as bass
import concourse.bass_isa as bass_isa
import concourse.mybir as mybir
import concourse.tile as tile

BF = mybir.dt.bfloat16
F32 = mybir.dt.float32

B, L_FULL, IN_DIM = 8, 2048, 5
D_MODEL, OUT_DIM = 64, 64
N_LAYERS, D_INNER, N_STATE, DT_RANK, K = 2, 128, 16, 4, 4
T = 2 * N_LAYERS
N_CORES = 8

MM_F = 512  # max matmul free dim (one PSUM bank of f32)


def _mm(nc, out, lhsT, rhs, start=True, stop=True):
    """matmul split into <=512-wide chunks along the moving free dim."""
    F = rhs.shape[-1]
    for j in range(0, F, MM_F):
        e = min(j + MM_F, F)
        nc.tensor.matmul(out[:, j:e], lhsT, rhs[:, j:e], start=start, stop=stop)


def build_nc(L=L_FULL, scan_engine="vector", yadd_split=True, silu_act=True):
    nc = bacc.Bacc("TRN2", target_bir_lowering=False)
    H = L // 2  # half length

    # ---------------- DRAM I/O ----------------
    d_xT = nc.dram_tensor("xT", (IN_DIM, L), F32, kind="ExternalInput")
    d_Wemb = nc.dram_tensor("Wemb", (IN_DIM, D_MODEL), F32, kind="ExternalInput")
    d_bemb = nc.dram_tensor("bemb", (D_MODEL, 1), F32, kind="ExternalInput")
    d_peT = nc.dram_tensor("peT", (D_MODEL, L), F32, kind="ExternalInput")
    d_Win = nc.dram_tensor("Win", (D_MODEL, T * 2 * D_INNER), BF, kind="ExternalInput")
    d_beta = nc.dram_tensor("beta", (D_INNER, 2 * T), F32, kind="ExternalInput")
    d_cdiag = nc.dram_tensor("cdiag", (D_INNER, T * K * D_INNER), BF, kind="ExternalInput")
    d_bconv = nc.dram_tensor("bconv", (D_INNER, T), F32, kind="ExternalInput")
    d_Wxdt = nc.dram_tensor("Wxdt", (D_INNER, T * DT_RANK), BF, kind="ExternalInput")
    d_Wdt = nc.dram_tensor("Wdt", (DT_RANK, T * D_INNER), BF, kind="ExternalInput")
    d_bdt = nc.dram_tensor("bdt", (D_INNER, T), F32, kind="ExternalInput")
    # replicated-column weights for the (group, state)-layout scan:
    # BmRep[d', l*128 + g*16+n] = W_x[l][d', 4+n]  (independent of g)
    d_WxB = nc.dram_tensor("WxB", (D_INNER, T * D_INNER), BF, kind="ExternalInput")
    d_WxC = nc.dram_tensor("WxC", (D_INNER, T * D_INNER), BF, kind="ExternalInput")
    # sel[d', s*128 + g*16+n] = 1 iff d' == 8s+g  (layer-independent)
    d_sel = nc.dram_tensor("sel", (D_INNER, N_STATE * D_INNER), mybir.dt.float32r,
                           kind="ExternalInput")
    d_selb = nc.dram_tensor("selb", (D_INNER, N_STATE * D_INNER), BF, kind="ExternalInput")
    # sum64[g*16+n, k*64 + (k*8+g)] = 1 for k = s%8 (y-reduce selector)
    d_sum8 = nc.dram_tensor("sum8", (D_INNER, 8 * 64), BF, kind="ExternalInput")
    # AcolRep[g*16+n, l*16+s] = A[l][g*16+s, n]
    d_Acol = nc.dram_tensor("Acol", (D_INNER, T * N_STATE), F32, kind="ExternalInput")
    d_Dsk = nc.dram_tensor("Dsk", (D_INNER, T), F32, kind="ExternalInput")
    d_Wout = nc.dram_tensor("Wout", (D_INNER, T * D_MODEL), BF, kind="ExternalInput")
    d_Wproj = nc.dram_tensor("Wproj", (2 * D_MODEL, OUT_DIM), F32, kind="ExternalInput")
    d_bproj = nc.dram_tensor("bproj", (OUT_DIM, 1), F32, kind="ExternalInput")
    d_out = nc.dram_tensor("out", (OUT_DIM, 1), F32, kind="ExternalOutput")

    with ExitStack() as ctx:
        tc = ctx.enter_context(tile.TileContext(nc))
        wp = ctx.enter_context(tc.tile_pool(name="weights", bufs=1))
        hp = ctx.enter_context(tc.tile_pool(name="hres", bufs=2))
        ap = ctx.enter_context(tc.tile_pool(name="acts", bufs=1))
        np_ = ctx.enter_context(tc.tile_pool(name="nloop", bufs=2))
        np3 = ctx.enter_context(tc.tile_pool(name="nloop3", bufs=3))

        # ---------------- load weights ----------------
        def wload(d, shape, dtype, nsplit=1):
            t = wp.tile(list(shape), dtype, tag="w_" + d.name)
            f = shape[1]
            step = (f + nsplit - 1) // nsplit
            for j in range(0, f, step):
                e = min(j + step, f)
                nc.sync.dma_start(t[:, j:e], d[:, j:e])
            return t

        s_Wemb = wload(d_Wemb, (IN_DIM, D_MODEL), F32)
        s_bemb = wload(d_bemb, (D_MODEL, 1), F32)
        s_Win = wload(d_Win, (D_MODEL, T * 2 * D_INNER), BF)
        s_beta = wload(d_beta, (D_INNER, 2 * T), F32)
        s_cdiag = wload(d_cdiag, (D_INNER, T * K * D_INNER), BF, nsplit=2)
        s_bconv = wload(d_bconv, (D_INNER, T), F32)
        s_Wxdt = wload(d_Wxdt, (D_INNER, T * DT_RANK), BF)
        s_Wdt = wload(d_Wdt, (DT_RANK, T * D_INNER), BF)
        s_bdt = wload(d_bdt, (D_INNER, T), F32)
        s_WxB = wload(d_WxB, (D_INNER, T * D_INNER), BF)
        s_WxC = wload(d_WxC, (D_INNER, T * D_INNER), BF)
        s_sel = wload(d_sel, (D_INNER, N_STATE * D_INNER), mybir.dt.float32r, nsplit=4)
        s_selb = wload(d_selb, (D_INNER, N_STATE * D_INNER), BF, nsplit=2)
        s_sum8 = wload(d_sum8, (D_INNER, 8 * 64), BF)
        s_Acol = wload(d_Acol, (D_INNER, T * N_STATE), F32)
        s_Dsk = wload(d_Dsk, (D_INNER, T), F32)
        s_Wout = wload(d_Wout, (D_INNER, T * D_MODEL), BF)
        s_Wproj = wload(d_Wproj, (2 * D_MODEL, OUT_DIM), F32)
        s_bproj = wload(d_bproj, (OUT_DIM, 1), F32)
        s_eps = wp.tile([D_MODEL, 1], F32)
        nc.vector.memset(s_eps[:], 1e-5)

        # ---------------- embedding ----------------
        with tc.tile_pool(name="ps_emb", bufs=2, space="PSUM") as pse, \
             tc.tile_pool(name="embin", bufs=1) as ep:
            s_xT = ep.tile([IN_DIM, L], F32, tag="xT")
            nc.sync.dma_start(s_xT[:], d_xT[:])
            s_peT = ep.tile([D_MODEL, L], F32, tag="peT")
            for j in (0, H):
                nc.sync.dma_start(s_peT[:, j:j + H], d_peT[:, j:j + H])
            h_f = hp.tile([D_MODEL, L], F32, tag="hf")
            for j in (0, H):
                eP = pse.tile([D_MODEL, H], F32, tag="emb")
                _mm(nc, eP, s_Wemb[:], s_xT[:, j:j + H])
                # h = (emb + bemb) + peT
                nc.vector.scalar_tensor_tensor(
                    h_f[:, j:j + H], eP[:], s_bemb[:], s_peT[:, j:j + H],
                    mybir.AluOpType.add, mybir.AluOpType.add)
        h_b = hp.tile([D_MODEL, L], F32, tag="hb")
        nc.vector.tensor_copy(h_b[:], h_f[:, ::-1])

        def act_silu(out_ap, in_psum, bias_ap, pool, tag, shape):
            """silu(x+b); real Silu table on HW, sigmoid+mul fallback for sim."""
            if silu_act:
                nc.scalar.activation(out_ap, in_psum,
                                     mybir.ActivationFunctionType.Silu, bias=bias_ap)
            else:
                sg = pool.tile(shape, F32, tag=tag + "_sg")
                nc.scalar.activation(sg[:], in_psum,
                                     mybir.ActivationFunctionType.Sigmoid, bias=bias_ap)
                zt = pool.tile(shape, F32, tag=tag + "_zt")
                nc.scalar.activation(zt[:], in_psum,
                                     mybir.ActivationFunctionType.Identity, bias=bias_ap)
                nc.vector.tensor_tensor(out_ap, zt[:], sg[:], mybir.AluOpType.mult)

        # ---------------- mamba layers ----------------
        def layer(l, h_in, htag):
            # ---- LN stats (over 64 partitions); lnA reused in place ----
            hn = ap.tile([D_MODEL, L], BF, tag="hn")
            for j in (0, H):
                hi_ = h_in[:, j:j + H]
                lnA = ap.tile([D_MODEL, H], F32, tag="lnA")
                lnB = ap.tile([D_MODEL, H], F32, tag="lnB")
                lnC = ap.tile([D_MODEL, H], F32, tag="lnC")
                nc.scalar.square(lnA[:], hi_)  # h^2
                nc.gpsimd.partition_all_reduce(lnB[:], hi_, channels=D_MODEL,
                                               reduce_op=bass_isa.ReduceOp.add)
                nc.gpsimd.partition_all_reduce(lnC[:], lnA[:], channels=D_MODEL,
                                               reduce_op=bass_isa.ReduceOp.add)
                nc.scalar.activation(lnA[:], lnB[:],
                                     mybir.ActivationFunctionType.Square,
                                     scale=1.0 / D_MODEL)  # mu^2
                nc.vector.scalar_tensor_tensor(lnA[:], lnC[:], 1.0 / D_MODEL, lnA[:],
                                               mybir.AluOpType.mult,
                                               mybir.AluOpType.subtract)  # var
                nc.scalar.activation(lnA[:], lnA[:], mybir.ActivationFunctionType.Ln,
                                     bias=s_eps[:])
                nc.scalar.activation(lnA[:], lnA[:], mybir.ActivationFunctionType.Exp,
                                     scale=-0.5)  # inv
                nc.vector.scalar_tensor_tensor(lnB[:], lnB[:], -1.0 / D_MODEL, hi_,
                                               mybir.AluOpType.mult,
                                               mybir.AluOpType.add)  # h - mu
                nc.vector.tensor_tensor(hn[:, j:j + H], lnB[:], lnA[:],
                                        mybir.AluOpType.mult)

            xi = ap.tile([D_INNER, L + K - 1], BF, tag="xi")
            nc.vector.memset(xi[:, 0:K - 1], 0.0)
            xc = ap.tile([D_INNER, L], BF, tag="xc" + htag)
            sz = ap.tile([D_INNER, L], BF, tag="sz" + htag)
            delta = ap.tile([D_INNER, L], mybir.dt.float32r, tag="delta" + htag)
            u = ap.tile([D_INNER, L], BF, tag="u" + htag)

            w_in = s_Win[:, l * 2 * D_INNER:(l + 1) * 2 * D_INNER]
            with tc.tile_pool(name=f"ps_mm{l}", bufs=2, space="PSUM") as psm:
                for j in (0, H):
                    # xi half
                    xiP = psm.tile([D_INNER, H], F32, tag="mm")
                    _mm(nc, xiP, w_in[:, 0:D_INNER], hn[:, j:j + H])
                    nc.scalar.activation(xi[:, K - 1 + j:K - 1 + j + H], xiP[:],
                                         mybir.ActivationFunctionType.Identity,
                                         bias=s_beta[:, 2 * l:2 * l + 1])
                    # z half -> silu
                    zP = psm.tile([D_INNER, H], F32, tag="mm")
                    _mm(nc, zP, w_in[:, D_INNER:2 * D_INNER], hn[:, j:j + H])
                    act_silu(sz[:, j:j + H], zP[:], s_beta[:, 2 * l + 1:2 * l + 2],
                             ap, "szf", [D_INNER, H])
                for j in (0, H):
                    # conv half: 4 accumulating diagonal matmuls over padded xi
                    cP = psm.tile([D_INNER, H], F32, tag="mm")
                    for k in range(K):
                        dg = s_cdiag[:, (l * K + k) * D_INNER:(l * K + k + 1) * D_INNER]
                        _mm(nc, cP, dg, xi[:, j + k:j + k + H],
                            start=(k == 0), stop=(k == K - 1))
                    act_silu(xc[:, j:j + H], cP[:], s_bconv[:, l:l + 1],
                             ap, "xcf", [D_INNER, H])
                # dt path
                dt_bf = ap.tile([DT_RANK, L], BF, tag="dtbf")
                for j in (0, H):
                    dtP = psm.tile([DT_RANK, H], F32, tag="mm")
                    _mm(nc, dtP, s_Wxdt[:, l * DT_RANK:(l + 1) * DT_RANK], xc[:, j:j + H])
                    nc.scalar.copy(dt_bf[:, j:j + H], dtP[:])
                for j in (0, H):
                    dpP = psm.tile([D_INNER, H], F32, tag="mm")
                    _mm(nc, dpP, s_Wdt[:, l * D_INNER:(l + 1) * D_INNER], dt_bf[:, j:j + H])
                    # softplus(x) = ln(exp(x) + 1) via the natural_log_exp table
                    dexp = ap.tile([D_INNER, H], F32, tag="scr128")
                    nc.scalar.activation(dexp[:], dpP[:],
                                         mybir.ActivationFunctionType.Exp,
                                         bias=s_bdt[:, l:l + 1])
                    nc.scalar.activation(delta[:, j:j + H], dexp[:],
                                         mybir.ActivationFunctionType.Ln,
                                         bias=1.0)
            nc.vector.tensor_tensor(u[:], delta[:].bitcast(F32), xc[:], mybir.AluOpType.mult)

            # ---- selective scan in (group, state) partition layout ----
            # Each of the 16 scan tiles handles d in {g*16+s : g in 0..7} with
            # partitions p = g*16+n.  Bm/Cm coefficient tiles are shared across
            # all 16 tiles (one matmul + one bf16 copy per layer each).
            rep_bf = {}
            with tc.tile_pool(name=f"ps_rep{l}", bufs=1, space="PSUM") as psr:
                for nm, w_all in (("bm", s_WxB), ("cm", s_WxC)):
                    rP = psr.tile([D_INNER, L], F32, tag="rep")
                    _mm(nc, rP, w_all[:, l * D_INNER:(l + 1) * D_INNER], xc[:, 0:L])
                    rb = ap.tile([D_INNER, L], BF, tag=nm + "rep")
                    nc.scalar.copy(rb[:], rP[:])
                    rep_bf[nm] = rb
            with tc.tile_pool(name=f"ps_bc{l}", bufs=1, space="PSUM") as psb, \
                 tc.tile_pool(name=f"ps_u{l}", bufs=1, space="PSUM") as psu, \
                 tc.tile_pool(name=f"ps_y{l}", bufs=1, space="PSUM") as psy:
                # all 16 scan tiles' n-reductions accumulate into one psum tile
                yacc = psy.tile([D_INNER, L], F32, tag="yacc")
                for s in range(N_STATE):
                    sel = s_sel[:, s * D_INNER:(s + 1) * D_INNER]
                    selb = s_selb[:, s * D_INNER:(s + 1) * D_INNER]
                    acol = s_Acol[:, l * N_STATE + s:l * N_STATE + s + 1]
                    # dA in replicated layout: delta row 8s+g -> partitions (g, :)
                    dA = np_.tile([D_INNER, L], F32, tag="dA")
                    for j in (0, H):
                        dP = psb.tile([D_INNER, H], F32, tag="drep")
                        _mm(nc, dP, sel, delta[:, j:j + H])
                        nc.scalar.activation(dA[:, j:j + H], dP[:],
                                             mybir.ActivationFunctionType.Exp,
                                             scale=acol)
                    # u in replicated layout via the same PE selector matmul
                    dBx = np3.tile([D_INNER, L], BF, tag="dBx")
                    for j in (0, H):
                        uP = psu.tile([D_INNER, H], F32, tag="urep")
                        _mm(nc, uP, selb, u[:, j:j + H])
                        nc.vector.scalar_tensor_tensor(
                            dBx[:, j:j + H], uP[:], 1.0, rep_bf["bm"][:, j:j + H],
                            mybir.AluOpType.bypass, mybir.AluOpType.mult)
                    hs = np3.tile([D_INNER, L], BF, tag="hs")
                    nc.vector.tensor_tensor_scan(
                        hs[:], dA[:], dBx[:], 0.0,
                        mybir.AluOpType.mult, mybir.AluOpType.add)
                    p = np3.tile([D_INNER, L], BF, tag="p")
                    peng = nc.gpsimd if s % 4 != 0 else nc.vector
                    peng.tensor_tensor(p[:], rep_bf["cm"][:], hs[:],
                                       mybir.AluOpType.mult)
                    # reduce over n (partition groups) on PE, accumulating 4
                    # consecutive s into each 32-partition block of yacc
                    k = s % 8
                    blk = (s // 8) * 64
                    for j in range(0, L, MM_F):
                        e = min(j + MM_F, L)
                        nc.tensor.matmul(yacc[blk:blk + 64, j:e],
                                         s_sum8[:, k * 64:(k + 1) * 64],
                                         p[:, j:e],
                                         start=(k == 0), stop=(k == 7),
                                         skip_group_check=True)

                # ---- postprocess (reads yacc psum) ----
                yg = ap.tile([D_INNER, L], BF, tag="yg")
                y2 = ap.tile([D_INNER, L], BF, tag="scr128")
                nc.vector.scalar_tensor_tensor(
                    y2[:], xc[:, 0:L], s_Dsk[:, l:l + 1], yacc[:, 0:L],
                    mybir.AluOpType.mult, mybir.AluOpType.add)
                nc.vector.tensor_tensor(yg[:], y2[:], sz[:],
                                        mybir.AluOpType.mult)
            h_out = hp.tile([D_MODEL, L], F32, tag=htag)
            with tc.tile_pool(name=f"ps_out{l}", bufs=2, space="PSUM") as pso:
                for j in (0, H):
                    oP = pso.tile([D_MODEL, H], F32, tag="out")
                    _mm(nc, oP, s_Wout[:, l * D_MODEL:(l + 1) * D_MODEL], yg[:, j:j + H])
                    nc.vector.tensor_tensor(h_out[:, j:j + H], h_in[:, j:j + H], oP[:],
                                            mybir.AluOpType.add)
            return h_out

        hf, hb = h_f, h_b
        for l in range(N_LAYERS):
            hf = layer(l, hf, "hf")
            hb = layer(N_LAYERS + l, hb, "hb")

        # ---------------- head ----------------
        mf = ap.tile([D_MODEL, 1], F32, tag="mf")
        nc.vector.tensor_reduce(mf[:], hf[:], axis=mybir.AxisListType.X,
                                op=mybir.AluOpType.add)
        mb = ap.tile([D_MODEL, 1], F32, tag="mb")
        nc.vector.tensor_reduce(mb[:], hb[:], axis=mybir.AxisListType.X,
                                op=mybir.AluOpType.add)
        zv = ap.tile([2 * D_MODEL, 1], F32, tag="zv")
        nc.sync.dma_start(zv[0:D_MODEL, :], mf[:])
        nc.sync.dma_start(zv[D_MODEL:2 * D_MODEL, :], mb[:])
        with tc.tile_pool(name="ps_head", bufs=1, space="PSUM") as psh:
            oP = psh.tile([OUT_DIM, 1], F32, tag="o")
            nc.tensor.matmul(oP[:], s_Wproj[:], zv[:])
            ofin = ap.tile([OUT_DIM, 1], F32, tag="ofin")
            nc.scalar.activation(ofin[:], oP[:], mybir.ActivationFunctionType.Identity,
                                 bias=s_bproj[:])
            nc.sync.dma_start(d_out[:], ofin[:])

    return nc


def prep_inputs(inputs, L=L_FULL):
    """Host-side packing of weights into the kernel's DRAM blobs."""
    bf = ml_dtypes.bfloat16
    f32 = np.float32
    g = {k: np.asarray(v) for k, v in inputs.items()}
    W_in, W_conv, W_x, W_dt = g["W_in"], g["W_conv"], g["W_x"], g["W_dt"]
    ln_w, ln_b = g["ln_w"], g["ln_b"]

    Win = np.concatenate([W_in[l] * ln_w[l][:, None] for l in range(T)], axis=1)
    beta = np.stack([ln_b[l] @ W_in[l] for l in range(T)], 0)  # (T, 256)
    beta_blob = np.zeros((D_INNER, 2 * T), f32)
    for l in range(T):
        beta_blob[:, 2 * l] = beta[l, :D_INNER]
        beta_blob[:, 2 * l + 1] = beta[l, D_INNER:]
    cdiag = np.zeros((D_INNER, T * K * D_INNER), f32)
    for l in range(T):
        for k in range(K):
            blk = (l * K + k) * D_INNER
            cdiag[np.arange(D_INNER), blk + np.arange(D_INNER)] = W_conv[l, :, 0, k]
    Wxdt = np.concatenate([W_x[l][:, :DT_RANK] for l in range(T)], axis=1)
    Wdt = np.concatenate([W_dt[l] for l in range(T)], axis=1)
    # (group, state)-layout blobs: partitions p = g*16+n, tile index s
    WxB = np.concatenate(
        [np.tile(W_x[l][:, DT_RANK:DT_RANK + N_STATE], (1, 8)) for l in range(T)],
        axis=1)
    WxC = np.concatenate(
        [np.tile(W_x[l][:, DT_RANK + N_STATE:], (1, 8)) for l in range(T)], axis=1)
    sel = np.zeros((D_INNER, N_STATE * D_INNER), f32)
    for s in range(N_STATE):
        for gg in range(8):
            sel[8 * s + gg, s * D_INNER + gg * 16:s * D_INNER + gg * 16 + 16] = 1.0
    sum8 = np.zeros((D_INNER, 8 * 64), f32)
    for k in range(8):
        for gg in range(8):
            sum8[gg * 16:(gg + 1) * 16, k * 64 + k * 8 + gg] = 1.0
    A = -np.exp(g["A_log"])  # (T, 128, 16)
    Acol = np.zeros((D_INNER, T * N_STATE), f32)
    for l in range(T):
        for s in range(N_STATE):
            Acol[:, l * N_STATE + s] = A[l][8 * s:8 * s + 8, :].reshape(-1)
    Wout = np.concatenate([g["W_out"][l] for l in range(T)], axis=1)

    shared = {
        "Wemb": g["W_emb"].astype(f32),
        "bemb": g["b_emb"].reshape(D_MODEL, 1).astype(f32),
        "peT": np.ascontiguousarray(g["pe"][:L].T).astype(f32),
        "Win": Win.astype(bf),
        "beta": beta_blob,
        "cdiag": cdiag.astype(bf),
        "bconv": np.ascontiguousarray(g["b_conv"].T).astype(f32),
        "Wxdt": Wxdt.astype(bf),
        "Wdt": Wdt.astype(bf),
        "bdt": np.ascontiguousarray(g["b_dt"].T).astype(f32),
        "WxB": WxB.astype(bf),
        "WxC": WxC.astype(bf),
        "sel": sel,
        "selb": sel.astype(bf),
        "sum8": sum8.astype(bf),
        "Acol": Acol.astype(f32),
        "Dsk": np.ascontiguousarray(g["D_skip"].T).astype(f32),
        "Wout": Wout.astype(bf),
        "Wproj": (g["W_proj"] / L).astype(f32),
        "bproj": g["b_proj"].reshape(OUT_DIM, 1).astype(f32),
    }
    in_maps = []
    for c in range(B):
        m = dict(shared)
        m["xT"] = np.ascontiguousarray(g["x"][c, :L].T).astype(f32)
        in_maps.append(m)
    return in_maps


_CACHE = {}


def kernel(**inputs):
    if "nc" not in _CACHE:
        _CACHE["nc"] = build_nc()
        _CACHE["nc"].finalize()
    nc = _CACHE["nc"]
    in_maps = prep_inputs(inputs)
    from concourse.bass_utils import run_bass_kernel_spmd
    res = run_bass_kernel_spmd(nc, in_maps, core_ids=list(range(N_CORES)))
    out = np.stack([np.asarray(res.results[c]["out"]).reshape(OUT_DIM)
                    for c in range(N_CORES)], axis=0)
    return out.astype(np.float32)



# revision 6
# speedup vs baseline: 1.2162x; 1.2162x over previous
"""Trainium2 Bass kernel for nn_MicroBiMambaBackbone.

Strategy: data-parallel over batch (B=8 -> 8 cores, 1 sample/core).
Per core, activations live in (feature-partition, L-free) layout.

v2 restructure vs v1:
  - The forward stack (layers 0,1) and backward stack (layers 2,3) are
    data-independent until the head.  Their phases are issued staggered
    (P(F0) S(F0) P(B0) O(F0) S(B0) P(F1) O(B0) S(F1) ...) so the
    vector engine runs scan phases back-to-back while PE/scalar do the
    other chain's layernorm/matmul prelude in the shadow.
  - LayerNorm stats via PE ones-matmul (frees gpsimd), Rsqrt table.
  - Causal conv fused into the input projection: 4 accumulating
    matmuls with host-prescaled weights over a 65-row hn (row 64 = 1
    carries the folded ln_b bias; 3 leading zero columns implement the
    causal pad), skipping the xi intermediate entirely.
  - dt path collapsed to one rank-4 (128x128) matmul + Softplus table.
  - One global (128,1024) PSUM transient pool (4 banks) + one
    (128,2048) yacc accumulator tag (4 banks).
"""

import sys

sys.path.insert(0, "/opt/trn_rl_repo")

from contextlib import ExitStack

import ml_dtypes
import numpy as np

import concourse.bacc as bacc
import concourse.bass as bass
import concourse.mybir as mybir
import concourse.tile as tile

BF = mybir.dt.bfloat16
F32 = mybir.dt.float32
F32R = mybir.dt.float32r

B, L, IN_DIM = 8, 2048, 5
D_MODEL, OUT_DIM = 64, 64
N_LAYERS, D_INNER, N_STATE, DT_RANK, K = 2, 128, 16, 4, 4
T = 2 * N_LAYERS
N_CORES = 8
H = L // 2
MM_F = 512

AF = mybir.ActivationFunctionType
OP = mybir.AluOpType


def _mm(nc, out, lhsT, rhs, start=True, stop=True):
    F = rhs.shape[-1]
    for j in range(0, F, MM_F):
        e = min(j + MM_F, F)
        nc.tensor.matmul(out[:, j:e], lhsT, rhs[:, j:e], start=start, stop=stop)


def build_nc():
    nc = bacc.Bacc("TRN2", target_bir_lowering=False)

    # ---------------- DRAM I/O ----------------
    d_xT = nc.dram_tensor("xT", (IN_DIM, L), BF, kind="ExternalInput")
    d_Wemb = nc.dram_tensor("Wemb", (IN_DIM, D_MODEL), BF, kind="ExternalInput")
    d_bemb = nc.dram_tensor("bemb", (D_MODEL, 1), F32, kind="ExternalInput")
    d_peT = nc.dram_tensor("peT", (D_MODEL, L), BF, kind="ExternalInput")
    d_ones = nc.dram_tensor("ones64", (D_MODEL, D_MODEL), F32, kind="ExternalInput")
    d_onesb = nc.dram_tensor("ones64b", (D_MODEL, D_MODEL), BF, kind="ExternalInput")
    d_WinK = nc.dram_tensor("WinK", (D_MODEL + 1, T * K * D_INNER), BF,
                            kind="ExternalInput")
    d_Wz = nc.dram_tensor("Wz", (D_MODEL + 1, T * D_INNER), BF, kind="ExternalInput")
    d_Wdtf = nc.dram_tensor("Wdtf", (D_INNER, T * D_INNER), BF, kind="ExternalInput")
    d_bdt = nc.dram_tensor("bdt", (D_INNER, T), F32, kind="ExternalInput")
    d_bconv = nc.dram_tensor("bconv", (D_INNER, T), F32, kind="ExternalInput")
    d_WxB = nc.dram_tensor("WxB", (D_INNER, T * D_INNER), BF, kind="ExternalInput")
    d_WxC = nc.dram_tensor("WxC", (D_INNER, T * D_INNER), BF, kind="ExternalInput")
    d_sel = nc.dram_tensor("sel", (D_INNER, N_STATE * D_INNER), F32R,
                           kind="ExternalInput")
    d_selb = nc.dram_tensor("selb", (D_INNER, N_STATE * D_INNER), BF,
                            kind="ExternalInput")
    d_sum8 = nc.dram_tensor("sum8", (D_INNER, 8 * 64), BF, kind="ExternalInput")
    d_Acol = nc.dram_tensor("Acol", (D_INNER, T * N_STATE), F32, kind="ExternalInput")
    d_Dsk = nc.dram_tensor("Dsk", (D_INNER, T), F32, kind="ExternalInput")
    d_Wout = nc.dram_tensor("Wout", (D_INNER, T * D_MODEL), BF, kind="ExternalInput")
    d_Wproj = nc.dram_tensor("Wproj", (2 * D_MODEL, OUT_DIM), F32,
                             kind="ExternalInput")
    d_bproj = nc.dram_tensor("bproj", (OUT_DIM, 1), F32, kind="ExternalInput")
    d_out = nc.dram_tensor("out", (OUT_DIM, 1), F32, kind="ExternalOutput")

    with ExitStack() as ctx:
        tc = ctx.enter_context(tile.TileContext(nc))
        wp = ctx.enter_context(tc.tile_pool(name="weights", bufs=1))
        hp = ctx.enter_context(tc.tile_pool(name="hres", bufs=2))
        ap = ctx.enter_context(tc.tile_pool(name="acts", bufs=1))
        sp2 = ctx.enter_context(tc.tile_pool(name="scan2", bufs=2))
        ps = ctx.enter_context(tc.tile_pool(name="ps", bufs=2, space="PSUM"))
        py = ctx.enter_context(tc.tile_pool(name="py", bufs=1, space="PSUM"))

        def wload(d, shape, dtype, nsplit=1):
            t = wp.tile(list(shape), dtype, tag="w_" + d.name)
            f = shape[1]
            step = (f + nsplit - 1) // nsplit
            for j in range(0, f, step):
                e = min(j + step, f)
                nc.sync.dma_start(t[:, j:e], d[:, j:e])
            return t

        # embedding-critical loads first
        s_xT = wload(d_xT, (IN_DIM, L), BF)
        s_Wemb = wload(d_Wemb, (IN_DIM, D_MODEL), BF)
        s_bemb = wload(d_bemb, (D_MODEL, 1), F32)
        s_peT = wload(d_peT, (D_MODEL, L), BF)
        # layer-0 prelude weights
        s_ones = wload(d_ones, (D_MODEL, D_MODEL), F32)
        s_onesb = wload(d_onesb, (D_MODEL, D_MODEL), BF)
        s_WinK = wload(d_WinK, (D_MODEL + 1, T * K * D_INNER), BF, nsplit=2)
        s_Wz = wload(d_Wz, (D_MODEL + 1, T * D_INNER), BF)
        s_Wdtf = wload(d_Wdtf, (D_INNER, T * D_INNER), BF)
        s_bdt = wload(d_bdt, (D_INNER, T), F32)
        s_bconv = wload(d_bconv, (D_INNER, T), F32)
        s_WxB = wload(d_WxB, (D_INNER, T * D_INNER), BF)
        s_WxC = wload(d_WxC, (D_INNER, T * D_INNER), BF)
        # scan-phase weights
        s_sel = wload(d_sel, (D_INNER, N_STATE * D_INNER), F32R, nsplit=4)
        s_selb = wload(d_selb, (D_INNER, N_STATE * D_INNER), BF, nsplit=2)
        s_sum8 = wload(d_sum8, (D_INNER, 8 * 64), BF)
        s_Acol = wload(d_Acol, (D_INNER, T * N_STATE), F32)
        s_Dsk = wload(d_Dsk, (D_INNER, T), F32)
        s_Wout = wload(d_Wout, (D_INNER, T * D_MODEL), BF)
        s_Wproj = wload(d_Wproj, (2 * D_MODEL, OUT_DIM), F32)
        s_bproj = wload(d_bproj, (OUT_DIM, 1), F32)
        s_eps = wp.tile([D_MODEL, 1], F32)
        nc.vector.memset(s_eps[:], 1e-5)

        # ---------------- embedding ----------------
        h_f = hp.tile([D_MODEL, L], F32, tag="hf")
        for j in (0, H):
            eP = ps.tile([D_INNER, H], F32, tag="ps")
            _mm(nc, eP[0:D_MODEL, :], s_Wemb[:], s_xT[:, j:j + H])
            nc.vector.scalar_tensor_tensor(
                h_f[:, j:j + H], eP[0:D_MODEL, :], s_bemb[:],
                s_peT[:, j:j + H], OP.add, OP.add)
        h_b = hp.tile([D_MODEL, L], F32, tag="hb")
        nc.vector.tensor_copy(h_b[:], h_f[:, ::-1])

        # ---------------- phase builders ----------------
        def prelude(l, ch, h_in):
            """LN + fused conv/in-proj + z + dt + rep; fills per-chain acts."""
            c_t = ap.tile([D_MODEL, L], BF, tag="lnc")
            inv = ap.tile([D_MODEL, L], BF, tag="lninv")
            for j in (0, H):
                mP = ps.tile([D_INNER, H], F32, tag="ps")
                _mm(nc, mP[0:D_MODEL, :], s_ones[:], h_in[:, j:j + H])
                nc.vector.scalar_tensor_tensor(
                    c_t[:, j:j + H], mP[0:D_MODEL, :], -1.0, h_in[:, j:j + H],
                    OP.mult, OP.add)  # c = h - mu
            sq = ap.tile([D_MODEL, L], BF, tag="lnsq")
            nc.scalar.activation(sq[:], c_t[:], AF.Square)
            for j in (0, H):
                vP = ps.tile([D_INNER, H], F32, tag="ps")
                _mm(nc, vP[0:D_MODEL, :], s_onesb[:], sq[:, j:j + H])
                nc.scalar.activation(inv[:, j:j + H], vP[0:D_MODEL, :],
                                     AF.Abs_reciprocal_sqrt, bias=s_eps[:])
            hn = ap.tile([D_MODEL + 1, L + K - 1], BF, tag="hn")
            nc.vector.memset(hn[0:D_MODEL + 1, 0:K - 1], 0.0)
            nc.vector.memset(hn[D_MODEL:D_MODEL + 1, K - 1:L + K - 1], 1.0)
            for j in (0, H):
                nc.vector.tensor_tensor(hn[0:D_MODEL, K - 1 + j:K - 1 + j + H],
                                        c_t[:, j:j + H], inv[:, j:j + H],
                                        OP.mult)

            xc = ap.tile([D_INNER, L], BF, tag="xc" + ch)
            sz = ap.tile([D_INNER, L], BF, tag="sz" + ch)
            delta = ap.tile([D_INNER, L], F32R, tag="delta" + ch)
            u = ap.tile([D_INNER, L], BF, tag="u" + ch)
            for j in (0, H):
                cP = ps.tile([D_INNER, H], F32, tag="ps")
                for k in range(K):
                    wk = s_WinK[:, (l * K + k) * D_INNER:(l * K + k + 1) * D_INNER]
                    _mm(nc, cP, wk, hn[:, k + j:k + j + H],
                        start=(k == 0), stop=(k == K - 1))
                nc.scalar.activation(xc[:, j:j + H], cP[:], AF.Silu,
                                     bias=s_bconv[:, l:l + 1])
            for j in (0, H):
                zP = ps.tile([D_INNER, H], F32, tag="ps")
                _mm(nc, zP, s_Wz[:, l * D_INNER:(l + 1) * D_INNER],
                    hn[:, K - 1 + j:K - 1 + j + H])
                nc.scalar.activation(sz[:, j:j + H], zP[:], AF.Silu)
            for j in (0, H):
                dP = ps.tile([D_INNER, H], F32, tag="ps")
                _mm(nc, dP, s_Wdtf[:, l * D_INNER:(l + 1) * D_INNER],
                    xc[:, j:j + H])
                # softplus(x) = ln(exp(x) + 1) via the natural_log_exp table
                dex = ap.tile([D_INNER, H], F32, tag="dex")
                nc.scalar.activation(dex[:], dP[:], AF.Exp,
                                     bias=s_bdt[:, l:l + 1])
                nc.scalar.activation(delta[:, j:j + H], dex[:],
                                     AF.Ln, bias=1.0)
            nc.vector.tensor_tensor(u[:], delta[:].bitcast(F32), xc[:], OP.mult)
            bm = ap.tile([D_INNER, L], BF, tag="bm" + ch)
            cm = ap.tile([D_INNER, L], BF, tag="cm" + ch)
            for nm, w_all in ((bm, s_WxB), (cm, s_WxC)):
                for j in (0, H):
                    rP = ps.tile([D_INNER, H], F32, tag="ps")
                    _mm(nc, rP, w_all[:, l * D_INNER:(l + 1) * D_INNER],
                        xc[:, j:j + H])
                    nc.scalar.copy(nm[:, j:j + H], rP[:])
            return dict(xc=xc, sz=sz, delta=delta, u=u, bm=bm, cm=cm)

        def scan_phase(l, acts):
            """16 (group,state)-layout scan tiles; returns yacc psum tile."""
            yacc = py.tile([D_INNER, L], F32, tag="yacc")
            delta, u, bm, cm = (acts["delta"], acts["u"], acts["bm"],
                                acts["cm"])
            for s in range(N_STATE):
                sel = s_sel[:, s * D_INNER:(s + 1) * D_INNER]
                selb = s_selb[:, s * D_INNER:(s + 1) * D_INNER]
                acol = s_Acol[:, l * N_STATE + s:l * N_STATE + s + 1]
                dA = sp2.tile([D_INNER, L], F32, tag="dA")
                for j in (0, H):
                    dpP = ps.tile([D_INNER, H], F32, tag="ps")
                    _mm(nc, dpP, sel, delta[:, j:j + H])
                    nc.scalar.activation(dA[:, j:j + H], dpP[:], AF.Exp,
                                         scale=acol)
                dBx = sp2.tile([D_INNER, L], BF, tag="dBx")
                for j in (0, H):
                    uP = ps.tile([D_INNER, H], F32, tag="ps")
                    _mm(nc, uP, selb, u[:, j:j + H])
                    nc.vector.scalar_tensor_tensor(
                        dBx[:, j:j + H], uP[:], 1.0, bm[:, j:j + H],
                        OP.bypass, OP.mult)
                hs = sp2.tile([D_INNER, L], BF, tag="hs")
                nc.vector.tensor_tensor_scan(hs[:], dA[:], dBx[:], 0.0,
                                             OP.mult, OP.add)
                p = sp2.tile([D_INNER, L], BF, tag="p")
                nc.gpsimd.tensor_tensor(p[:], cm[:], hs[:], OP.mult)
                k = s % 8
                blk = (s // 8) * 64
                for j in range(0, L, MM_F):
                    e = min(j + MM_F, L)
                    nc.tensor.matmul(yacc[blk:blk + 64, j:e],
                                     s_sum8[:, k * 64:(k + 1) * 64],
                                     p[:, j:e],
                                     start=(k == 0), stop=(k == 7),
                                     skip_group_check=True)
            return yacc

        def outphase(l, ch, h_in, yacc, acts):
            y2 = ap.tile([D_INNER, L], BF, tag="y2")
            for j in (0, H):
                nc.vector.scalar_tensor_tensor(
                    y2[:, j:j + H], acts["xc"][:, j:j + H],
                    s_Dsk[:, l:l + 1], yacc[:, j:j + H], OP.mult, OP.add)
            yg = ap.tile([D_INNER, L], BF, tag="yg")
            nc.vector.tensor_tensor(yg[:], y2[:], acts["sz"][:], OP.mult)
            h_out = hp.tile([D_MODEL, L], F32, tag="h" + ch)
            for j in (0, H):
                oP = ps.tile([D_INNER, H], F32, tag="ps")
                _mm(nc, oP[0:D_MODEL, :],
                    s_Wout[:, l * D_MODEL:(l + 1) * D_MODEL], yg[:, j:j + H])
                nc.vector.scalar_tensor_tensor(
                    h_out[:, j:j + H], oP[0:D_MODEL, :], 1.0,
                    h_in[:, j:j + H], OP.bypass, OP.add)
            return h_out

        # ---------------- staggered schedule ----------------
        # F chain: layers 0,1 on h_f ; B chain: layers 2,3 on h_b
        aF = prelude(0, "f", h_f)
        yF = scan_phase(0, aF)
        aB = prelude(2, "b", h_b)
        h_f = outphase(0, "f", h_f, yF, aF)
        yB = scan_phase(2, aB)
        aF = prelude(1, "f", h_f)
        h_b = outphase(2, "b", h_b, yB, aB)
        yF = scan_phase(1, aF)
        aB = prelude(3, "b", h_b)
        h_f = outphase(1, "f", h_f, yF, aF)
        mf = ap.tile([D_MODEL, 1], F32, tag="mf")
        nc.vector.tensor_reduce(mf[:], h_f[:], axis=mybir.AxisListType.X,
                                op=OP.add)
        yB = scan_phase(3, aB)
        h_b = outphase(3, "b", h_b, yB, aB)

        # ---------------- head ----------------
        mb = ap.tile([D_MODEL, 1], F32, tag="mb")
        nc.vector.tensor_reduce(mb[:], h_b[:], axis=mybir.AxisListType.X,
                                op=OP.add)
        zv = ap.tile([2 * D_MODEL, 1], F32, tag="zv")
        nc.sync.dma_start(zv[0:D_MODEL, :], mf[:])
        nc.sync.dma_start(zv[D_MODEL:2 * D_MODEL, :], mb[:])
        oP = ps.tile([D_INNER, H], F32, tag="ps")
        nc.tensor.matmul(oP[0:OUT_DIM, 0:1], s_Wproj[:], zv[:])
        ofin = ap.tile([OUT_DIM, 1], F32, tag="ofin")
        nc.scalar.activation(ofin[:], oP[0:OUT_DIM, 0:1], AF.Identity,
                             bias=s_bproj[:])
        nc.sync.dma_start(d_out[:], ofin[:])

    return nc


def prep_inputs(inputs):
    bf = ml_dtypes.bfloat16
    f32 = np.float32
    g = {k: np.asarray(v) for k, v in inputs.items()}
    W_in, W_conv, W_x, W_dt = g["W_in"], g["W_conv"], g["W_x"], g["W_dt"]
    ln_w, ln_b = g["ln_w"], g["ln_b"]

    WinK = np.zeros((D_MODEL + 1, T * K * D_INNER), f32)
    Wz = np.zeros((D_MODEL + 1, T * D_INNER), f32)
    for l in range(T):
        Wl = W_in[l] * ln_w[l][:, None]          # (64, 256)
        bl = ln_b[l] @ W_in[l]                   # (256,)
        for k in range(K):
            blk = (l * K + k) * D_INNER
            wc = W_conv[l, :, 0, k]              # (128,)
            WinK[:D_MODEL, blk:blk + D_INNER] = Wl[:, :D_INNER] * wc[None, :]
            WinK[D_MODEL, blk:blk + D_INNER] = bl[:D_INNER] * wc
        Wz[:D_MODEL, l * D_INNER:(l + 1) * D_INNER] = Wl[:, D_INNER:]
        Wz[D_MODEL, l * D_INNER:(l + 1) * D_INNER] = bl[D_INNER:]
    Wdtf = np.concatenate(
        [W_x[l][:, :DT_RANK] @ W_dt[l] for l in range(T)], axis=1)
    WxB = np.concatenate(
        [np.tile(W_x[l][:, DT_RANK:DT_RANK + N_STATE], (1, 8))
         for l in range(T)], axis=1)
    WxC = np.concatenate(
        [np.tile(W_x[l][:, DT_RANK + N_STATE:], (1, 8)) for l in range(T)],
        axis=1)
    sel = np.zeros((D_INNER, N_STATE * D_INNER), f32)
    for s in range(N_STATE):
        for gg in range(8):
            sel[8 * s + gg, s * D_INNER + gg * 16:s * D_INNER + gg * 16 + 16] = 1.0
    sum8 = np.zeros((D_INNER, 8 * 64), f32)
    for k in range(8):
        for gg in range(8):
            sum8[gg * 16:(gg + 1) * 16, k * 64 + k * 8 + gg] = 1.0
    A = -np.exp(g["A_log"])
    Acol = np.zeros((D_INNER, T * N_STATE), f32)
    for l in range(T):
        for s in range(N_STATE):
            Acol[:, l * N_STATE + s] = A[l][8 * s:8 * s + 8, :].reshape(-1)
    Wout = np.concatenate([g["W_out"][l] for l in range(T)], axis=1)

    shared = {
        "Wemb": g["W_emb"].astype(bf),
        "bemb": g["b_emb"].reshape(D_MODEL, 1).astype(f32),
        "peT": np.ascontiguousarray(g["pe"][:L].T).astype(bf),
        "ones64": np.full((D_MODEL, D_MODEL), 1.0 / D_MODEL, f32),
        "ones64b": np.full((D_MODEL, D_MODEL), 1.0 / D_MODEL, bf),
        "WinK": WinK.astype(bf),
        "Wz": Wz.astype(bf),
        "Wdtf": Wdtf.astype(bf),
        "bdt": np.ascontiguousarray(g["b_dt"].T).astype(f32),
        "bconv": np.ascontiguousarray(g["b_conv"].T).astype(f32),
        "WxB": WxB.astype(bf),
        "WxC": WxC.astype(bf),
        "sel": sel,
        "selb": sel.astype(bf),
        "sum8": sum8.astype(bf),
        "Acol": Acol.astype(f32),
        "Dsk": np.ascontiguousarray(g["D_skip"].T).astype(f32),
        "Wout": Wout.astype(bf),
        "Wproj": (g["W_proj"] / L).astype(f32),
        "bproj": g["b_proj"].reshape(OUT_DIM, 1).astype(f32),
    }
    in_maps = []
    for c in range(B):
        m = dict(shared)
        m["xT"] = np.ascontiguousarray(g["x"][c, :L].T).astype(bf)
        in_maps.append(m)
    return in_maps


_CACHE = {}


def kernel(**inputs):
    if "nc" not in _CACHE:
        _CACHE["nc"] = build_nc()
        _CACHE["nc"].finalize()
    nc = _CACHE["nc"]
    in_maps = prep_inputs(inputs)
    from concourse.bass_utils import run_bass_kernel_spmd
    res = run_bass_kernel_spmd(nc, in_maps, core_ids=list(range(N_CORES)))
    out = np.stack([np.asarray(res.results[c]["out"]).reshape(OUT_DIM)
                    for c in range(N_CORES)], axis=0)
    return out.astype(np.float32)


# revision 8
# speedup vs baseline: 1.3278x; 1.0918x over previous
"""Trainium2 Bass kernel for nn_MicroBiMambaBackbone.

Strategy: data-parallel over batch (B=8 -> 8 cores, 1 sample/core).
Per core, activations live in (feature-partition, L-free) layout.

v3:
  - Forward (layers 0,1) and backward (layers 2,3) chains staggered;
    each prelude's instructions are interleaved INTO the other chain's
    scan-tile loop so PE/scalar work lands in the scan shadow (engine
    queues are in-order, so issue order controls overlap).
  - All elementwise scan-phase work on Vector (gpsimd contends with
    the DVE SBUF port and stalls tensor_tensor_scan ~1:1, so it is
    left idle).
  - dBx path: uP psum is copied to SBUF bf16 by Scalar (which
    otherwise idles waiting on PE), making dBx / p both all-bf16
    SBUF->SBUF 2x-mode vector TTs.
  - delta in bf16: the dP replication matmul reuses the bf16 selector,
    no f32r weights at all.
  - LayerNorm stats via PE ones-matmuls; 1/sqrt via the
    abs_reciprocal_sqrt table.
  - Causal conv fused into the input projection (65-row hn with a
    persistent ones row and 3 zero pad columns, host-prescaled taps).
  - dt path collapsed to one rank-4 (128x128) matmul; softplus via the
    shared natural_log_exp table (same table as the scan-phase exps).
  - Head means folded into the last residual adds via STT accum_out.
"""

import sys

sys.path.insert(0, "/opt/trn_rl_repo")

from contextlib import ExitStack

import ml_dtypes
import numpy as np

import concourse.bacc as bacc
import concourse.bass as bass
import concourse.mybir as mybir
import concourse.tile as tile

BF = mybir.dt.bfloat16
F32 = mybir.dt.float32

B, L, IN_DIM = 8, 2048, 5
D_MODEL, OUT_DIM = 64, 64
N_LAYERS, D_INNER, N_STATE, DT_RANK, K = 2, 128, 16, 4, 4
T = 2 * N_LAYERS
N_CORES = 8
H = L // 2
MM_F = 512

AF = mybir.ActivationFunctionType
OP = mybir.AluOpType


def _mm(nc, out, lhsT, rhs, start=True, stop=True):
    F = rhs.shape[-1]
    for j in range(0, F, MM_F):
        e = min(j + MM_F, F)
        nc.tensor.matmul(out[:, j:e], lhsT, rhs[:, j:e], start=start, stop=stop)


def build_nc():
    nc = bacc.Bacc("TRN2", target_bir_lowering=False)

    # ---------------- DRAM I/O ----------------
    d_xT = nc.dram_tensor("xT", (IN_DIM, L), BF, kind="ExternalInput")
    d_Wemb = nc.dram_tensor("Wemb", (IN_DIM, D_MODEL), BF, kind="ExternalInput")
    d_bemb = nc.dram_tensor("bemb", (D_MODEL, 1), F32, kind="ExternalInput")
    d_peT = nc.dram_tensor("peT", (D_MODEL, L), BF, kind="ExternalInput")
    d_ones = nc.dram_tensor("ones64", (D_MODEL, D_MODEL), F32, kind="ExternalInput")
    d_onesb = nc.dram_tensor("ones64b", (D_MODEL, D_MODEL), BF, kind="ExternalInput")
    d_WinK = nc.dram_tensor("WinK", (D_MODEL + 1, T * K * D_INNER), BF,
                            kind="ExternalInput")
    d_Wz = nc.dram_tensor("Wz", (D_MODEL + 1, T * D_INNER), BF, kind="ExternalInput")
    d_Wdtf = nc.dram_tensor("Wdtf", (D_INNER, T * D_INNER), BF, kind="ExternalInput")
    d_bdt = nc.dram_tensor("bdt", (D_INNER, T), F32, kind="ExternalInput")
    d_bconv = nc.dram_tensor("bconv", (D_INNER, T), F32, kind="ExternalInput")
    d_WxB = nc.dram_tensor("WxB", (D_INNER, T * D_INNER), BF, kind="ExternalInput")
    d_WxC = nc.dram_tensor("WxC", (D_INNER, T * D_INNER), BF, kind="ExternalInput")
    d_selb = nc.dram_tensor("selb", (D_INNER, N_STATE * D_INNER), BF,
                            kind="ExternalInput")
    d_sum8 = nc.dram_tensor("sum8", (D_INNER, 8 * 64), BF, kind="ExternalInput")
    d_Acol = nc.dram_tensor("Acol", (D_INNER, T * N_STATE), F32, kind="ExternalInput")
    d_Dsk = nc.dram_tensor("Dsk", (D_INNER, T), F32, kind="ExternalInput")
    d_Wout = nc.dram_tensor("Wout", (D_INNER, T * D_MODEL), BF, kind="ExternalInput")
    d_Wproj = nc.dram_tensor("Wproj", (2 * D_MODEL, OUT_DIM), F32,
                             kind="ExternalInput")
    d_bproj = nc.dram_tensor("bproj", (OUT_DIM, 1), F32, kind="ExternalInput")
    d_out = nc.dram_tensor("out", (OUT_DIM, 1), F32, kind="ExternalOutput")

    with ExitStack() as ctx:
        tc = ctx.enter_context(tile.TileContext(nc))
        wp = ctx.enter_context(tc.tile_pool(name="weights", bufs=1))
        hp = ctx.enter_context(tc.tile_pool(name="hres", bufs=2))
        ap = ctx.enter_context(tc.tile_pool(name="acts", bufs=1))
        sp2 = ctx.enter_context(tc.tile_pool(name="scan2", bufs=2))
        ps = ctx.enter_context(tc.tile_pool(name="ps", bufs=2, space="PSUM"))
        py = ctx.enter_context(tc.tile_pool(name="py", bufs=1, space="PSUM"))

        def wload(d, shape, dtype, nsplit=1):
            t = wp.tile(list(shape), dtype, tag="w_" + d.name)
            f = shape[1]
            step = (f + nsplit - 1) // nsplit
            for j in range(0, f, step):
                e = min(j + step, f)
                nc.sync.dma_start(t[:, j:e], d[:, j:e])
            return t

        # embedding-critical loads first
        s_xT = wload(d_xT, (IN_DIM, L), BF)
        s_Wemb = wload(d_Wemb, (IN_DIM, D_MODEL), BF)
        s_bemb = wload(d_bemb, (D_MODEL, 1), F32)
        s_peT = wload(d_peT, (D_MODEL, L), BF)
        # layer-0 prelude weights
        s_ones = wload(d_ones, (D_MODEL, D_MODEL), F32)
        s_onesb = wload(d_onesb, (D_MODEL, D_MODEL), BF)
        s_WinK = wload(d_WinK, (D_MODEL + 1, T * K * D_INNER), BF, nsplit=2)
        s_Wz = wload(d_Wz, (D_MODEL + 1, T * D_INNER), BF)
        s_Wdtf = wload(d_Wdtf, (D_INNER, T * D_INNER), BF)
        s_bdt = wload(d_bdt, (D_INNER, T), F32)
        s_bconv = wload(d_bconv, (D_INNER, T), F32)
        s_WxB = wload(d_WxB, (D_INNER, T * D_INNER), BF)
        s_WxC = wload(d_WxC, (D_INNER, T * D_INNER), BF)
        # scan-phase weights
        s_selb = wload(d_selb, (D_INNER, N_STATE * D_INNER), BF, nsplit=2)
        s_sum8 = wload(d_sum8, (D_INNER, 8 * 64), BF)
        s_Acol = wload(d_Acol, (D_INNER, T * N_STATE), F32)
        s_Dsk = wload(d_Dsk, (D_INNER, T), F32)
        s_Wout = wload(d_Wout, (D_INNER, T * D_MODEL), BF)
        s_Wproj = wload(d_Wproj, (2 * D_MODEL, OUT_DIM), F32)
        s_bproj = wload(d_bproj, (OUT_DIM, 1), F32)
        s_eps = wp.tile([D_MODEL, 1], F32)
        nc.vector.memset(s_eps[:], 1e-5)
        # persistent 65-row hn buffer: ones row + zero pad set once
        hn = wp.tile([D_MODEL + 1, L + K - 1], BF)
        nc.vector.memset(hn[0:D_MODEL + 1, 0:K - 1], 0.0)
        nc.vector.memset(hn[D_MODEL:D_MODEL + 1, K - 1:L + K - 1], 1.0)

        # ---------------- embedding ----------------
        h_f = hp.tile([D_MODEL, L], F32, tag="hf")
        for j in (0, H):
            eP = ps.tile([D_INNER, H], F32, tag="ps")
            _mm(nc, eP[0:D_MODEL, :], s_Wemb[:], s_xT[:, j:j + H])
            nc.vector.scalar_tensor_tensor(
                h_f[:, j:j + H], eP[0:D_MODEL, :], s_bemb[:],
                s_peT[:, j:j + H], OP.add, OP.add)
        h_b = hp.tile([D_MODEL, L], F32, tag="hb")
        nc.vector.tensor_copy(h_b[:], h_f[:, ::-1])

        # ---------------- phase builders ----------------
        def prelude_gen(l, ch, h_in, acts):
            """LN + fused conv/in-proj + z + dt + rep, as a generator so the
            scan loop of the other chain can interleave its issue order."""
            c_t = ap.tile([D_MODEL, L], BF, tag="lnc")
            inv = ap.tile([D_MODEL, L], BF, tag="lninv")
            for j in (0, H):
                mP = ps.tile([D_INNER, H], F32, tag="ps")
                _mm(nc, mP[0:D_MODEL, :], s_ones[:], h_in[:, j:j + H])
                nc.vector.scalar_tensor_tensor(
                    c_t[:, j:j + H], mP[0:D_MODEL, :], -1.0, h_in[:, j:j + H],
                    OP.mult, OP.add)  # c = h - mu
            yield
            sq = ap.tile([D_MODEL, L], BF, tag="lnsq")
            nc.scalar.activation(sq[:], c_t[:], AF.Square)
            for j in (0, H):
                vP = ps.tile([D_INNER, H], F32, tag="ps")
                _mm(nc, vP[0:D_MODEL, :], s_onesb[:], sq[:, j:j + H])
                nc.scalar.activation(inv[:, j:j + H], vP[0:D_MODEL, :],
                                     AF.Abs_reciprocal_sqrt, bias=s_eps[:])
            yield
            for j in (0, H):
                nc.vector.tensor_tensor(hn[0:D_MODEL, K - 1 + j:K - 1 + j + H],
                                        c_t[:, j:j + H], inv[:, j:j + H],
                                        OP.mult)
            yield
            xc = acts["xc"] = ap.tile([D_INNER, L], BF, tag="xc" + ch, name="xc")
            sz = acts["sz"] = ap.tile([D_INNER, L], BF, tag="sz" + ch, name="sz")
            delta = acts["delta"] = ap.tile([D_INNER, L], BF, tag="delta" + ch, name="delta")
            u = acts["u"] = ap.tile([D_INNER, L], BF, tag="u" + ch, name="u")
            for j in (0, H):
                cP = ps.tile([D_INNER, H], F32, tag="ps")
                for k in range(K):
                    wk = s_WinK[:, (l * K + k) * D_INNER:(l * K + k + 1) * D_INNER]
                    _mm(nc, cP, wk, hn[:, k + j:k + j + H],
                        start=(k == 0), stop=(k == K - 1))
                nc.scalar.activation(xc[:, j:j + H], cP[:], AF.Silu,
                                     bias=s_bconv[:, l:l + 1])
                yield
            for j in (0, H):
                zP = ps.tile([D_INNER, H], F32, tag="ps")
                _mm(nc, zP, s_Wz[:, l * D_INNER:(l + 1) * D_INNER],
                    hn[:, K - 1 + j:K - 1 + j + H])
                nc.scalar.activation(sz[:, j:j + H], zP[:], AF.Silu)
            yield
            for j in (0, H):
                dP = ps.tile([D_INNER, H], F32, tag="ps")
                _mm(nc, dP, s_Wdtf[:, l * D_INNER:(l + 1) * D_INNER],
                    xc[:, j:j + H])
                # softplus(x) = ln(exp(x) + 1) via the natural_log_exp table
                dex = ap.tile([D_INNER, H], F32, tag="dex")
                nc.scalar.activation(dex[:], dP[:], AF.Exp,
                                     bias=s_bdt[:, l:l + 1])
                nc.scalar.activation(delta[:, j:j + H], dex[:], AF.Ln,
                                     bias=1.0)
                yield
            nc.vector.tensor_tensor(u[:], delta[:], xc[:], OP.mult)
            yield
            bm = acts["bm"] = ap.tile([D_INNER, L], BF, tag="bm" + ch, name="bm")
            cm = acts["cm"] = ap.tile([D_INNER, L], BF, tag="cm" + ch, name="cm")
            for nm, w_all in ((bm, s_WxB), (cm, s_WxC)):
                for j in (0, H):
                    rP = ps.tile([D_INNER, H], F32, tag="ps")
                    _mm(nc, rP, w_all[:, l * D_INNER:(l + 1) * D_INNER],
                        xc[:, j:j + H])
                    nc.scalar.copy(nm[:, j:j + H], rP[:])
                yield

        def scan_phase(l, acts, shadow=None):
            """16 (group,state)-layout scan tiles; returns yacc psum tile.
            After each tile, one step of `shadow` (the other chain's prelude
            generator) is issued so its PE/scalar work overlaps the scans."""
            yacc = py.tile([D_INNER, L], F32, tag="yacc")
            delta, u, bm, cm = (acts["delta"], acts["u"], acts["bm"],
                                acts["cm"])
            for s in range(N_STATE):
                selb = s_selb[:, s * D_INNER:(s + 1) * D_INNER]
                acol = s_Acol[:, l * N_STATE + s:l * N_STATE + s + 1]
                dA = sp2.tile([D_INNER, L], F32, tag="dA")
                urep = sp2.tile([D_INNER, L], BF, tag="urep")
                for j in (0, H):
                    dpP = ps.tile([D_INNER, H], F32, tag="ps")
                    _mm(nc, dpP, selb, delta[:, j:j + H])
                    nc.scalar.activation(dA[:, j:j + H], dpP[:], AF.Exp,
                                         scale=acol)
                for j in (0, H):
                    uP = ps.tile([D_INNER, H], F32, tag="ps")
                    _mm(nc, uP, selb, u[:, j:j + H])
                    nc.scalar.copy(urep[:, j:j + H], uP[:])
                dBx = sp2.tile([D_INNER, L], BF, tag="dBx")
                nc.vector.tensor_tensor(dBx[:], urep[:], bm[:], OP.mult)
                hs = sp2.tile([D_INNER, L], BF, tag="hs")
                nc.vector.tensor_tensor_scan(hs[:], dA[:], dBx[:], 0.0,
                                             OP.mult, OP.add)
                p = sp2.tile([D_INNER, L], BF, tag="p")
                nc.vector.tensor_tensor(p[:], cm[:], hs[:], OP.mult)
                k = s % 8
                blk = (s // 8) * 64
                for j in range(0, L, MM_F):
                    e = min(j + MM_F, L)
                    nc.tensor.matmul(yacc[blk:blk + 64, j:e],
                                     s_sum8[:, k * 64:(k + 1) * 64],
                                     p[:, j:e],
                                     start=(k == 0), stop=(k == 7),
                                     skip_group_check=True)
                if shadow is not None:
                    next(shadow, None)
            if shadow is not None:
                for _ in shadow:
                    pass
            return yacc

        def outphase(l, ch, h_in, yacc, acts, macc=None):
            y2 = ap.tile([D_INNER, L], BF, tag="y2")
            for j in (0, H):
                nc.vector.scalar_tensor_tensor(
                    y2[:, j:j + H], acts["xc"][:, j:j + H],
                    s_Dsk[:, l:l + 1], yacc[:, j:j + H], OP.mult, OP.add)
            yg = ap.tile([D_INNER, L], BF, tag="yg")
            nc.vector.tensor_tensor(yg[:], y2[:], acts["sz"][:], OP.mult)
            h_out = hp.tile([D_MODEL, L], F32, tag="h" + ch)
            for ji, j in enumerate((0, H)):
                oP = ps.tile([D_INNER, H], F32, tag="ps")
                _mm(nc, oP[0:D_MODEL, :],
                    s_Wout[:, l * D_MODEL:(l + 1) * D_MODEL], yg[:, j:j + H])
                nc.vector.scalar_tensor_tensor(
                    h_out[:, j:j + H], oP[0:D_MODEL, :], 1.0,
                    h_in[:, j:j + H], OP.bypass, OP.add,
                    accum_out=None if macc is None else macc[:, ji:ji + 1])
            return h_out

        # ---------------- staggered schedule ----------------
        # F chain: layers 0,1 on h_f ; B chain: layers 2,3 on h_b
        aF, aB = {}, {}
        for _ in prelude_gen(0, "f", h_f, aF):
            pass
        gB = prelude_gen(2, "b", h_b, aB)
        yF = scan_phase(0, aF, shadow=gB)
        h_f = outphase(0, "f", h_f, yF, aF)
        aF2 = {}
        gF = prelude_gen(1, "f", h_f, aF2)
        yB = scan_phase(2, aB, shadow=gF)
        h_b = outphase(2, "b", h_b, yB, aB)
        aB2 = {}
        gB = prelude_gen(3, "b", h_b, aB2)
        yF = scan_phase(1, aF2, shadow=gB)
        maccf = ap.tile([D_MODEL, 2], F32, tag="maccf")
        h_f = outphase(1, "f", h_f, yF, aF2, macc=maccf)
        yB = scan_phase(3, aB2)
        maccb = ap.tile([D_MODEL, 2], F32, tag="maccb")
        h_b = outphase(3, "b", h_b, yB, aB2, macc=maccb)

        # ---------------- head ----------------
        mf = ap.tile([D_MODEL, 1], F32, tag="mf")
        nc.vector.tensor_tensor(mf[:], maccf[:, 0:1], maccf[:, 1:2], OP.add)
        mb = ap.tile([D_MODEL, 1], F32, tag="mb")
        nc.vector.tensor_tensor(mb[:], maccb[:, 0:1], maccb[:, 1:2], OP.add)
        zv = ap.tile([2 * D_MODEL, 1], F32, tag="zv")
        nc.sync.dma_start(zv[0:D_MODEL, :], mf[:])
        nc.sync.dma_start(zv[D_MODEL:2 * D_MODEL, :], mb[:])
        oP = ps.tile([D_INNER, H], F32, tag="ps")
        nc.tensor.matmul(oP[0:OUT_DIM, 0:1], s_Wproj[:], zv[:])
        ofin = ap.tile([OUT_DIM, 1], F32, tag="ofin")
        nc.scalar.activation(ofin[:], oP[0:OUT_DIM, 0:1], AF.Identity,
                             bias=s_bproj[:])
        nc.sync.dma_start(d_out[:], ofin[:])

    return nc


def prep_inputs(inputs):
    bf = ml_dtypes.bfloat16
    f32 = np.float32
    g = {k: np.asarray(v) for k, v in inputs.items()}
    W_in, W_conv, W_x, W_dt = g["W_in"], g["W_conv"], g["W_x"], g["W_dt"]
    ln_w, ln_b = g["ln_w"], g["ln_b"]

    WinK = np.zeros((D_MODEL + 1, T * K * D_INNER), f32)
    Wz = np.zeros((D_MODEL + 1, T * D_INNER), f32)
    for l in range(T):
        Wl = W_in[l] * ln_w[l][:, None]          # (64, 256)
        bl = ln_b[l] @ W_in[l]                   # (256,)
        for k in range(K):
            blk = (l * K + k) * D_INNER
            wc = W_conv[l, :, 0, k]              # (128,)
            WinK[:D_MODEL, blk:blk + D_INNER] = Wl[:, :D_INNER] * wc[None, :]
            WinK[D_MODEL, blk:blk + D_INNER] = bl[:D_INNER] * wc
        Wz[:D_MODEL, l * D_INNER:(l + 1) * D_INNER] = Wl[:, D_INNER:]
        Wz[D_MODEL, l * D_INNER:(l + 1) * D_INNER] = bl[D_INNER:]
    Wdtf = np.concatenate(
        [W_x[l][:, :DT_RANK] @ W_dt[l] for l in range(T)], axis=1)
    WxB = np.concatenate(
        [np.tile(W_x[l][:, DT_RANK:DT_RANK + N_STATE], (1, 8))
         for l in range(T)], axis=1)
    WxC = np.concatenate(
        [np.tile(W_x[l][:, DT_RANK + N_STATE:], (1, 8)) for l in range(T)],
        axis=1)
    sel = np.zeros((D_INNER, N_STATE * D_INNER), f32)
    for s in range(N_STATE):
        for gg in range(8):
            sel[8 * s + gg, s * D_INNER + gg * 16:s * D_INNER + gg * 16 + 16] = 1.0
    sum8 = np.zeros((D_INNER, 8 * 64), f32)
    for k in range(8):
        for gg in range(8):
            sum8[gg * 16:(gg + 1) * 16, k * 64 + k * 8 + gg] = 1.0
    A = -np.exp(g["A_log"])
    Acol = np.zeros((D_INNER, T * N_STATE), f32)
    for l in range(T):
        for s in range(N_STATE):
            Acol[:, l * N_STATE + s] = A[l][8 * s:8 * s + 8, :].reshape(-1)
    Wout = np.concatenate([g["W_out"][l] for l in range(T)], axis=1)

    shared = {
        "Wemb": g["W_emb"].astype(bf),
        "bemb": g["b_emb"].reshape(D_MODEL, 1).astype(f32),
        "peT": np.ascontiguousarray(g["pe"][:L].T).astype(bf),
        "ones64": np.full((D_MODEL, D_MODEL), 1.0 / D_MODEL, f32),
        "ones64b": np.full((D_MODEL, D_MODEL), 1.0 / D_MODEL, bf),
        "WinK": WinK.astype(bf),
        "Wz": Wz.astype(bf),
        "Wdtf": Wdtf.astype(bf),
        "bdt": np.ascontiguousarray(g["b_dt"].T).astype(f32),
        "bconv": np.ascontiguousarray(g["b_conv"].T).astype(f32),
        "WxB": WxB.astype(bf),
        "WxC": WxC.astype(bf),
        "selb": sel.astype(bf),
        "sum8": sum8.astype(bf),
        "Acol": Acol.astype(f32),
        "Dsk": np.ascontiguousarray(g["D_skip"].T).astype(f32),
        "Wout": Wout.astype(bf),
        "Wproj": (g["W_proj"] / L).astype(f32),
        "bproj": g["b_proj"].reshape(OUT_DIM, 1).astype(f32),
    }
    in_maps = []
    for c in range(B):
        m = dict(shared)
        m["xT"] = np.ascontiguousarray(g["x"][c, :L].T).astype(bf)
        in_maps.append(m)
    return in_maps


_CACHE = {}


def kernel(**inputs):
    if "nc" not in _CACHE:
        _CACHE["nc"] = build_nc()
        _CACHE["nc"].finalize()
    nc = _CACHE["nc"]
    in_maps = prep_inputs(inputs)
    from concourse.bass_utils import run_bass_kernel_spmd
    res = run_bass_kernel_spmd(nc, in_maps, core_ids=list(range(N_CORES)))
    out = np.stack([np.asarray(res.results[c]["out"]).reshape(OUT_DIM)
                    for c in range(N_CORES)], axis=0)
    return out.astype(np.float32)


# revision 12
# speedup vs baseline: 1.3712x; 1.0327x over previous
"""Trainium2 Bass kernel for nn_MicroBiMambaBackbone.

Strategy: data-parallel over batch (B=8 -> 8 cores, 1 sample/core).
Per core, activations live in (feature-partition, L-free) layout.

v3:
  - Forward (layers 0,1) and backward (layers 2,3) chains staggered;
    each prelude's instructions are interleaved INTO the other chain's
    scan-tile loop so PE/scalar work lands in the scan shadow (engine
    queues are in-order, so issue order controls overlap).
  - All elementwise scan-phase work on Vector (gpsimd contends with
    the DVE SBUF port and stalls tensor_tensor_scan ~1:1, so it is
    left idle).
  - dBx path: uP psum is copied to SBUF bf16 by Scalar (which
    otherwise idles waiting on PE), making dBx / p both all-bf16
    SBUF->SBUF 2x-mode vector TTs.
  - delta in bf16: the dP replication matmul reuses the bf16 selector,
    no f32r weights at all.
  - LayerNorm stats via PE ones-matmuls; 1/sqrt via the
    abs_reciprocal_sqrt table.
  - Causal conv fused into the input projection (65-row hn with a
    persistent ones row and 3 zero pad columns, host-prescaled taps).
  - dt path collapsed to one rank-4 (128x128) matmul; softplus via the
    shared natural_log_exp table (same table as the scan-phase exps).
  - Head means folded into the last residual adds via STT accum_out.
"""

import sys

sys.path.insert(0, "/opt/trn_rl_repo")

from contextlib import ExitStack

import ml_dtypes
import numpy as np

import concourse.bacc as bacc
import concourse.bass as bass
import concourse.mybir as mybir
import concourse.tile as tile

BF = mybir.dt.bfloat16
F32 = mybir.dt.float32

B, L, IN_DIM = 8, 2048, 5
D_MODEL, OUT_DIM = 64, 64
N_LAYERS, D_INNER, N_STATE, DT_RANK, K = 2, 128, 16, 4, 4
T = 2 * N_LAYERS
N_CORES = 8
H = L // 2
MM_F = 512

AF = mybir.ActivationFunctionType
OP = mybir.AluOpType


def _mm(nc, out, lhsT, rhs, start=True, stop=True):
    F = rhs.shape[-1]
    for j in range(0, F, MM_F):
        e = min(j + MM_F, F)
        nc.tensor.matmul(out[:, j:e], lhsT, rhs[:, j:e], start=start, stop=stop)


def build_nc():
    nc = bacc.Bacc("TRN2", target_bir_lowering=False)

    # ---------------- DRAM I/O ----------------
    d_xT = nc.dram_tensor("xT", (IN_DIM, L), BF, kind="ExternalInput")
    d_Wemb = nc.dram_tensor("Wemb", (IN_DIM, D_MODEL), BF, kind="ExternalInput")
    d_bemb = nc.dram_tensor("bemb", (D_MODEL, 1), F32, kind="ExternalInput")
    d_peT = nc.dram_tensor("peT", (D_MODEL, L), BF, kind="ExternalInput")
    d_ones = nc.dram_tensor("ones64", (D_MODEL, D_MODEL), F32, kind="ExternalInput")
    d_onesb = nc.dram_tensor("ones64b", (D_MODEL, D_MODEL), BF, kind="ExternalInput")
    d_WinK = nc.dram_tensor("WinK", (D_MODEL + 1, T * K * D_INNER), BF,
                            kind="ExternalInput")
    d_Wz = nc.dram_tensor("Wz", (D_MODEL + 1, T * D_INNER), BF, kind="ExternalInput")
    d_Wdtf = nc.dram_tensor("Wdtf", (D_INNER, T * D_INNER), BF, kind="ExternalInput")
    d_bdt = nc.dram_tensor("bdt", (D_INNER, T), F32, kind="ExternalInput")
    d_bconv = nc.dram_tensor("bconv", (D_INNER, T), F32, kind="ExternalInput")
    d_WxB = nc.dram_tensor("WxB", (D_INNER, T * D_INNER), BF, kind="ExternalInput")
    d_WxC = nc.dram_tensor("WxC", (D_INNER, T * D_INNER), BF, kind="ExternalInput")
    d_selb = nc.dram_tensor("selb", (D_INNER, N_STATE * D_INNER), BF,
                            kind="ExternalInput")
    d_sum8 = nc.dram_tensor("sum8", (D_INNER, 8 * 64), BF, kind="ExternalInput")
    d_Acol = nc.dram_tensor("Acol", (D_INNER, T * N_STATE), F32, kind="ExternalInput")
    d_Dsk = nc.dram_tensor("Dsk", (D_INNER, T), F32, kind="ExternalInput")
    d_Wout = nc.dram_tensor("Wout", (D_INNER, T * D_MODEL), BF, kind="ExternalInput")
    d_Wproj = nc.dram_tensor("Wproj", (D_MODEL, 2 * OUT_DIM), F32,
                             kind="ExternalInput")
    d_bproj = nc.dram_tensor("bproj", (OUT_DIM, 1), F32, kind="ExternalInput")
    d_out = nc.dram_tensor("out", (OUT_DIM, 1), F32, kind="ExternalOutput")

    with ExitStack() as ctx:
        tc = ctx.enter_context(tile.TileContext(nc))
        wp = ctx.enter_context(tc.tile_pool(name="weights", bufs=1))
        hp = ctx.enter_context(tc.tile_pool(name="hres", bufs=2))
        ap = ctx.enter_context(tc.tile_pool(name="acts", bufs=1))
        sp2 = ctx.enter_context(tc.tile_pool(name="scan2", bufs=2))
        ps = ctx.enter_context(tc.tile_pool(name="ps", bufs=2, space="PSUM"))
        py = ctx.enter_context(tc.tile_pool(name="py", bufs=1, space="PSUM"))

        def wload(d, shape, dtype, nsplit=1):
            t = wp.tile(list(shape), dtype, tag="w_" + d.name)
            f = shape[1]
            step = (f + nsplit - 1) // nsplit
            for j in range(0, f, step):
                e = min(j + step, f)
                nc.sync.dma_start(t[:, j:e], d[:, j:e])
            return t

        # embedding-critical loads first
        s_xT = wload(d_xT, (IN_DIM, L), BF)
        s_Wemb = wload(d_Wemb, (IN_DIM, D_MODEL), BF)
        s_bemb = wload(d_bemb, (D_MODEL, 1), F32)
        s_peT = wload(d_peT, (D_MODEL, L), BF)
        # layer-0 prelude weights
        s_ones = wload(d_ones, (D_MODEL, D_MODEL), F32)
        s_onesb = wload(d_onesb, (D_MODEL, D_MODEL), BF)
        s_WinK = wload(d_WinK, (D_MODEL + 1, T * K * D_INNER), BF, nsplit=2)
        s_Wz = wload(d_Wz, (D_MODEL + 1, T * D_INNER), BF)
        s_Wdtf = wload(d_Wdtf, (D_INNER, T * D_INNER), BF)
        s_bdt = wload(d_bdt, (D_INNER, T), F32)
        s_bconv = wload(d_bconv, (D_INNER, T), F32)
        s_WxB = wload(d_WxB, (D_INNER, T * D_INNER), BF)
        s_WxC = wload(d_WxC, (D_INNER, T * D_INNER), BF)
        # scan-phase weights
        s_selb = wload(d_selb, (D_INNER, N_STATE * D_INNER), BF, nsplit=2)
        s_sum8 = wload(d_sum8, (D_INNER, 8 * 64), BF)
        s_Acol = wload(d_Acol, (D_INNER, T * N_STATE), F32)
        s_Dsk = wload(d_Dsk, (D_INNER, T), F32)
        s_Wout = wload(d_Wout, (D_INNER, T * D_MODEL), BF)
        s_Wproj = wload(d_Wproj, (D_MODEL, 2 * OUT_DIM), F32)
        s_bproj = wload(d_bproj, (OUT_DIM, 1), F32)
        s_eps = wp.tile([D_MODEL, 1], F32)
        nc.vector.memset(s_eps[:], 1e-5)
        # persistent 65-row hn buffer: ones row + zero pad set once
        hn = wp.tile([D_MODEL + 1, L + K - 1], BF)
        nc.vector.memset(hn[0:D_MODEL + 1, 0:K - 1], 0.0)
        nc.vector.memset(hn[D_MODEL:D_MODEL + 1, K - 1:L + K - 1], 1.0)

        # ---------------- embedding ----------------
        h_f = hp.tile([D_MODEL, L], F32, tag="hf")
        for j in (0, H):
            eP = ps.tile([D_INNER, H], F32, tag="ps")
            _mm(nc, eP[0:D_MODEL, :], s_Wemb[:], s_xT[:, j:j + H])
            nc.vector.scalar_tensor_tensor(
                h_f[:, j:j + H], eP[0:D_MODEL, :], s_bemb[:],
                s_peT[:, j:j + H], OP.add, OP.add)
        h_b = hp.tile([D_MODEL, L], F32, tag="hb")
        nc.vector.tensor_copy(h_b[:], h_f[:, ::-1])

        # ---------------- phase builders ----------------
        def prelude_gen(l, ch, h_in, acts):
            """LN + fused conv/in-proj + z + dt + rep, as a generator so the
            scan loop of the other chain can interleave its issue order."""
            c_t = ap.tile([D_MODEL, L], BF, tag="lnc")
            inv = ap.tile([D_MODEL, L], BF, tag="lninv")
            for j in (0, H):
                mP = ps.tile([D_INNER, H], F32, tag="ps")
                _mm(nc, mP[0:D_MODEL, :], s_ones[:], h_in[:, j:j + H])
                nc.vector.scalar_tensor_tensor(
                    c_t[:, j:j + H], mP[0:D_MODEL, :], -1.0, h_in[:, j:j + H],
                    OP.mult, OP.add)  # c = h - mu
            yield
            # A-block: Square (available in every table) + both rsqrt halves
            # issued contiguously so the abs_reciprocal_sqrt table loads once.
            sq = ap.tile([D_MODEL, L], BF, tag="lnsq")
            nc.scalar.activation(sq[:], c_t[:], AF.Square)
            vPs = []
            for j in (0, H):
                vP = ps.tile([D_INNER, H], F32, tag="ps")
                _mm(nc, vP[0:D_MODEL, :], s_onesb[:], sq[:, j:j + H])
                vPs.append(vP)
            for ji, j in enumerate((0, H)):
                nc.scalar.activation(inv[:, j:j + H], vPs[ji][0:D_MODEL, :],
                                     AF.Abs_reciprocal_sqrt, bias=s_eps[:])
            yield
            for j in (0, H):
                nc.vector.tensor_tensor(hn[0:D_MODEL, K - 1 + j:K - 1 + j + H],
                                        c_t[:, j:j + H], inv[:, j:j + H],
                                        OP.mult)
            yield
            xc = acts["xc"] = ap.tile([D_INNER, L], BF, tag="xc" + ch, name="xc")
            sz = acts["sz"] = ap.tile([D_INNER, L], BF, tag="sz" + ch, name="sz")
            delta = acts["delta"] = ap.tile([D_INNER, L], BF, tag="delta" + ch, name="delta")
            u = acts["u"] = ap.tile([D_INNER, L], BF, tag="u" + ch, name="u")
            # S-block: conv + z matmuls with their four Silu consumes kept in
            # one uninterrupted step so the silu table loads once per layer.
            for j in (0, H):
                cP = ps.tile([D_INNER, H], F32, tag="ps")
                for k in range(K):
                    wk = s_WinK[:, (l * K + k) * D_INNER:(l * K + k + 1) * D_INNER]
                    _mm(nc, cP, wk, hn[:, k + j:k + j + H],
                        start=(k == 0), stop=(k == K - 1))
                nc.scalar.activation(xc[:, j:j + H], cP[:], AF.Silu,
                                     bias=s_bconv[:, l:l + 1])
            for j in (0, H):
                zP = ps.tile([D_INNER, H], F32, tag="ps")
                _mm(nc, zP, s_Wz[:, l * D_INNER:(l + 1) * D_INNER],
                    hn[:, K - 1 + j:K - 1 + j + H])
                nc.scalar.activation(sz[:, j:j + H], zP[:], AF.Silu)
            yield
            for j in (0, H):
                dP = ps.tile([D_INNER, H], F32, tag="ps")
                _mm(nc, dP, s_Wdtf[:, l * D_INNER:(l + 1) * D_INNER],
                    xc[:, j:j + H])
                # softplus(x) = ln(exp(x) + 1) via the natural_log_exp table
                dex = ap.tile([D_INNER, H], F32, tag="dex")
                nc.scalar.activation(dex[:], dP[:], AF.Exp,
                                     bias=s_bdt[:, l:l + 1])
                nc.scalar.activation(delta[:, j:j + H], dex[:], AF.Ln,
                                     bias=1.0)
                yield
            nc.vector.tensor_tensor(u[:], delta[:], xc[:], OP.mult)
            yield
            bm = acts["bm"] = ap.tile([D_INNER, L], BF, tag="bm" + ch, name="bm")
            cm = acts["cm"] = ap.tile([D_INNER, L], BF, tag="cm" + ch, name="cm")
            for nm, w_all in ((bm, s_WxB), (cm, s_WxC)):
                for j in (0, H):
                    rP = ps.tile([D_INNER, H], F32, tag="ps")
                    _mm(nc, rP, w_all[:, l * D_INNER:(l + 1) * D_INNER],
                        xc[:, j:j + H])
                    nc.scalar.copy(nm[:, j:j + H], rP[:])
                yield

        def scan_phase(l, acts, shadow=None):
            """16 (group,state)-layout scan tiles; returns yacc psum tile.
            After each tile, one step of `shadow` (the other chain's prelude
            generator) is issued so its PE/scalar work overlaps the scans."""
            yacc = py.tile([D_INNER, L], F32, tag="yacc")
            delta, u, bm, cm = (acts["delta"], acts["u"], acts["bm"],
                                acts["cm"])
            for s in range(N_STATE):
                selb = s_selb[:, s * D_INNER:(s + 1) * D_INNER]
                acol = s_Acol[:, l * N_STATE + s:l * N_STATE + s + 1]
                dA = sp2.tile([D_INNER, L], F32, tag="dA")
                urep = sp2.tile([D_INNER, L], BF, tag="urep")
                for j in (0, H):
                    dpP = ps.tile([D_INNER, H], F32, tag="ps")
                    _mm(nc, dpP, selb, delta[:, j:j + H])
                    nc.scalar.activation(dA[:, j:j + H], dpP[:], AF.Exp,
                                         scale=acol)
                for j in (0, H):
                    uP = ps.tile([D_INNER, H], F32, tag="ps")
                    _mm(nc, uP, selb, u[:, j:j + H])
                    nc.scalar.copy(urep[:, j:j + H], uP[:])
                dBx = sp2.tile([D_INNER, L], BF, tag="dBx")
                nc.vector.tensor_tensor(dBx[:], urep[:], bm[:], OP.mult)
                hs = sp2.tile([D_INNER, L], BF, tag="hs")
                nc.vector.tensor_tensor_scan(hs[:], dA[:], dBx[:], 0.0,
                                             OP.mult, OP.add)
                p = sp2.tile([D_INNER, L], BF, tag="p")
                nc.vector.tensor_tensor(p[:], cm[:], hs[:], OP.mult)
                k = s % 8
                blk = (s // 8) * 64
                for j in range(0, L, MM_F):
                    e = min(j + MM_F, L)
                    nc.tensor.matmul(yacc[blk:blk + 64, j:e],
                                     s_sum8[:, k * 64:(k + 1) * 64],
                                     p[:, j:e],
                                     start=(k == 0), stop=(k == 7),
                                     skip_group_check=True)
                if shadow is not None:
                    next(shadow, None)
            if shadow is not None:
                for _ in shadow:
                    pass
            return yacc

        def outphase(l, ch, h_in, yacc, acts, macc=None):
            y2 = ap.tile([D_INNER, L], BF, tag="y2")
            for j in (0, H):
                nc.vector.scalar_tensor_tensor(
                    y2[:, j:j + H], acts["xc"][:, j:j + H],
                    s_Dsk[:, l:l + 1], yacc[:, j:j + H], OP.mult, OP.add)
            yg = ap.tile([D_INNER, L], BF, tag="yg")
            nc.vector.tensor_tensor(yg[:], y2[:], acts["sz"][:], OP.mult)
            h_out = hp.tile([D_MODEL, L], F32, tag="h" + ch)
            for ji, j in enumerate((0, H)):
                oP = ps.tile([D_INNER, H], F32, tag="ps")
                _mm(nc, oP[0:D_MODEL, :],
                    s_Wout[:, l * D_MODEL:(l + 1) * D_MODEL], yg[:, j:j + H])
                nc.vector.scalar_tensor_tensor(
                    h_out[:, j:j + H], oP[0:D_MODEL, :], 1.0,
                    h_in[:, j:j + H], OP.bypass, OP.add,
                    accum_out=None if macc is None else macc[:, ji:ji + 1])
            return h_out

        # ---------------- staggered schedule ----------------
        # F chain: layers 0,1 on h_f ; B chain: layers 2,3 on h_b
        aF, aB = {}, {}
        for _ in prelude_gen(0, "f", h_f, aF):
            pass
        gB = prelude_gen(2, "b", h_b, aB)
        yF = scan_phase(0, aF, shadow=gB)
        h_f = outphase(0, "f", h_f, yF, aF)
        aF2 = {}
        gF = prelude_gen(1, "f", h_f, aF2)
        yB = scan_phase(2, aB, shadow=gF)
        h_b = outphase(2, "b", h_b, yB, aB)
        aB2 = {}
        gB = prelude_gen(3, "b", h_b, aB2)
        yF = scan_phase(1, aF2, shadow=gB)
        maccf = ap.tile([D_MODEL, 2], F32, tag="maccf")
        h_f = outphase(1, "f", h_f, yF, aF2, macc=maccf)
        yB = scan_phase(3, aB2)
        maccb = ap.tile([D_MODEL, 2], F32, tag="maccb")
        h_b = outphase(3, "b", h_b, yB, aB2, macc=maccb)

        # ---------------- head ----------------
        mf = ap.tile([D_MODEL, 1], F32, tag="mf")
        nc.vector.tensor_tensor(mf[:], maccf[:, 0:1], maccf[:, 1:2], OP.add)
        mb = ap.tile([D_MODEL, 1], F32, tag="mb")
        nc.vector.tensor_tensor(mb[:], maccb[:, 0:1], maccb[:, 1:2], OP.add)
        oP = ps.tile([D_INNER, H], F32, tag="ps")
        nc.tensor.matmul(oP[0:OUT_DIM, 0:1], s_Wproj[:, 0:OUT_DIM], mf[:],
                         start=True, stop=False)
        nc.tensor.matmul(oP[0:OUT_DIM, 0:1], s_Wproj[:, OUT_DIM:2 * OUT_DIM],
                         mb[:], start=False, stop=True)
        ofin = ap.tile([OUT_DIM, 1], F32, tag="ofin")
        nc.scalar.activation(ofin[:], oP[0:OUT_DIM, 0:1], AF.Identity,
                             bias=s_bproj[:])
        nc.sync.dma_start(d_out[:], ofin[:])

    return nc


def prep_inputs(inputs):
    bf = ml_dtypes.bfloat16
    f32 = np.float32
    g = {k: np.asarray(v) for k, v in inputs.items()}
    W_in, W_conv, W_x, W_dt = g["W_in"], g["W_conv"], g["W_x"], g["W_dt"]
    ln_w, ln_b = g["ln_w"], g["ln_b"]

    WinK = np.zeros((D_MODEL + 1, T * K * D_INNER), f32)
    Wz = np.zeros((D_MODEL + 1, T * D_INNER), f32)
    for l in range(T):
        Wl = W_in[l] * ln_w[l][:, None]          # (64, 256)
        bl = ln_b[l] @ W_in[l]                   # (256,)
        for k in range(K):
            blk = (l * K + k) * D_INNER
            wc = W_conv[l, :, 0, k]              # (128,)
            WinK[:D_MODEL, blk:blk + D_INNER] = Wl[:, :D_INNER] * wc[None, :]
            WinK[D_MODEL, blk:blk + D_INNER] = bl[:D_INNER] * wc
        Wz[:D_MODEL, l * D_INNER:(l + 1) * D_INNER] = Wl[:, D_INNER:]
        Wz[D_MODEL, l * D_INNER:(l + 1) * D_INNER] = bl[D_INNER:]
    Wdtf = np.concatenate(
        [W_x[l][:, :DT_RANK] @ W_dt[l] for l in range(T)], axis=1)
    WxB = np.concatenate(
        [np.tile(W_x[l][:, DT_RANK:DT_RANK + N_STATE], (1, 8))
         for l in range(T)], axis=1)
    WxC = np.concatenate(
        [np.tile(W_x[l][:, DT_RANK + N_STATE:], (1, 8)) for l in range(T)],
        axis=1)
    sel = np.zeros((D_INNER, N_STATE * D_INNER), f32)
    for s in range(N_STATE):
        for gg in range(8):
            sel[8 * s + gg, s * D_INNER + gg * 16:s * D_INNER + gg * 16 + 16] = 1.0
    sum8 = np.zeros((D_INNER, 8 * 64), f32)
    for k in range(8):
        for gg in range(8):
            sum8[gg * 16:(gg + 1) * 16, k * 64 + k * 8 + gg] = 1.0
    A = -np.exp(g["A_log"])
    Acol = np.zeros((D_INNER, T * N_STATE), f32)
    for l in range(T):
        for s in range(N_STATE):
            Acol[:, l * N_STATE + s] = A[l][8 * s:8 * s + 8, :].reshape(-1)
    Wout = np.concatenate([g["W_out"][l] for l in range(T)], axis=1)

    shared = {
        "Wemb": g["W_emb"].astype(bf),
        "bemb": g["b_emb"].reshape(D_MODEL, 1).astype(f32),
        "peT": np.ascontiguousarray(g["pe"][:L].T).astype(bf),
        "ones64": np.full((D_MODEL, D_MODEL), 1.0 / D_MODEL, f32),
        "ones64b": np.full((D_MODEL, D_MODEL), 1.0 / D_MODEL, bf),
        "WinK": WinK.astype(bf),
        "Wz": Wz.astype(bf),
        "Wdtf": Wdtf.astype(bf),
        "bdt": np.ascontiguousarray(g["b_dt"].T).astype(f32),
        "bconv": np.ascontiguousarray(g["b_conv"].T).astype(f32),
        "WxB": WxB.astype(bf),
        "WxC": WxC.astype(bf),
        "selb": sel.astype(bf),
        "sum8": sum8.astype(bf),
        "Acol": Acol.astype(f32),
        "Dsk": np.ascontiguousarray(g["D_skip"].T).astype(f32),
        "Wout": Wout.astype(bf),
        "Wproj": np.concatenate([(g["W_proj"][:D_MODEL] / L),
                                 (g["W_proj"][D_MODEL:] / L)],
                                axis=1).astype(f32),
        "bproj": g["b_proj"].reshape(OUT_DIM, 1).astype(f32),
    }
    in_maps = []
    for c in range(B):
        m = dict(shared)
        m["xT"] = np.ascontiguousarray(g["x"][c, :L].T).astype(bf)
        in_maps.append(m)
    return in_maps


_CACHE = {}


def kernel(**inputs):
    if "nc" not in _CACHE:
        _CACHE["nc"] = build_nc()
        _CACHE["nc"].finalize()
    nc = _CACHE["nc"]
    in_maps = prep_inputs(inputs)
    from concourse.bass_utils import run_bass_kernel_spmd
    res = run_bass_kernel_spmd(nc, in_maps, core_ids=list(range(N_CORES)))
    out = np.stack([np.asarray(res.results[c]["out"]).reshape(OUT_DIM)
                    for c in range(N_CORES)], axis=0)
    return out.astype(np.float32)
